# revision 55
# baseline (speedup 1.0000x reference)
"""Windowed cross-attention (sparse_attention) on Trainium2.

Data-parallel over the batch axis across 8 NeuronCores; each core processes
16 windows (4096 tokens) of the B=128 batch. Projections run in float32r
(full PE rate); the attention core (q*k logits, exp, attn*v) runs in bf16
inputs with fp32 PSUM accumulation. Host pre-transposes x/y to feature-major
layout and pre-bakes the relative-position bias per head pair so the device
program is pure matmul + softmax with no on-device transposes or gathers:

  qT = (q_w.T @ xT) * scale            (feature-major, written zero-padded)
  kT = kv_w[:, :C].T @ yT              (feature-major)
  v  = yT.T-tiles @ kv_w[:, C:]        (token-major, + 64 ones columns)
  attnT[k, (kt,hh,q)] = I.T @ biasT  +  kT.T @ qz   (one 512-col matmul each)
  expT = exp(attnT)                    (one 1024-col ACT op per head pair)
  ops = [v | 1s].T @ expT   -> rows 0:64 = unnormalized outT,
                               rows 64:128 = softmax denominator (x64)
  outT = ops[0:64] * reciprocal(ops[64:128])  (DVE only, no broadcasts)
  finT = proj_w.T-slices @ outT + proj_b      (bias via ACT Identity)

Head pairs (2j, 2j+1) occupy partition halves 0:64 / 64:128 of feature tile
j. The moving q tensor is stored block-diagonal ([128, 2, SBT] with the
off-diagonal blocks zero, pre-zeroed once in persistent buffers) so a single
128-contraction matmul computes both heads' logits into disjoint column
halves. x/y/out use a packed [128, 4, T] layout (one DMA each per
super-batch); the ones columns live in persistent v buffers written once at
module start. The projection work of super-batch sb+1 is interleaved at
head-pair granularity into the attention phase of super-batch sb so the
tensor engine always has independent work while ACT (exp) and DVE
(normalize) catch up.
"""

import numpy as np

_TRN_REPO = "/opt/trn_rl_repo"
N_CORES = 8
B, NW, C = 128, 256, 512        # full batch, window tokens, channels
H, D = 8, 64                    # heads, head dim
WH = WW = 16
BC = B // N_CORES               # windows per core
T = BC * NW                     # tokens per core
NSB_FULL = 8                    # super-batches (2 windows each) per core
SBT = T // NSB_FULL             # tokens per super-batch


def build_module(reps=1, mm="float32r", nsb=NSB_FULL, variant="full"):
    """Build + compile the per-core Bass module (SPMD; same program all cores)."""
    import sys
    if _TRN_REPO not in sys.path:
        sys.path.insert(0, _TRN_REPO)
    from contextlib import ExitStack

    import concourse.bacc as bacc
    import concourse.tile as tile
    from concourse import mybir

    f32 = mybir.dt.float32
    bf16 = mybir.dt.bfloat16
    mmdt = getattr(mybir.dt, mm)
    AF = mybir.ActivationFunctionType

    nc = bacc.Bacc("TRN2", debug=False, enable_asserts=False, num_devices=N_CORES)
    xT_d = nc.dram_tensor("xT", [128, 4, T], bf16, kind="ExternalInput")
    yT_d = nc.dram_tensor("yT", [128, 4, T], bf16, kind="ExternalInput")
    qw_d = nc.dram_tensor("qw", [C, C], bf16, kind="ExternalInput")
    kvw_d = nc.dram_tensor("kvw", [C, 2 * C], bf16, kind="ExternalInput")
    pw_d = nc.dram_tensor("pw", [C, C], mmdt, kind="ExternalInput")
    pbT_d = nc.dram_tensor("pbT", [128, 4], f32, kind="ExternalInput")
    # paired exp-bias: ebT[j, :, kt] = exp(bias)[128 k-rows,
    #                                           256q(head 2j) | 256q(head 2j+1)]
    ebT_d = nc.dram_tensor("ebT", [H // 2, 128, 2, 2 * NW], bf16,
                           kind="ExternalInput")
    outT_d = nc.dram_tensor("outT", [128, 4, T], bf16, kind="ExternalOutput")

    xT, yT, outT = xT_d.ap(), yT_d.ap(), outT_d.ap()

    with tile.TileContext(nc) as tc, ExitStack() as ctx:
        ctx.enter_context(nc.allow_low_precision(
            reason="attention core in bf16; projections f32r; fp32 accumulation"))
        consts = ctx.enter_context(tc.tile_pool(name="consts", bufs=1))
        xy_pool = ctx.enter_context(tc.tile_pool(name="xy", bufs=4))
        k_pool = ctx.enter_context(tc.tile_pool(name="kp", bufs=2))
        esin_pool = ctx.enter_context(tc.tile_pool(name="esin", bufs=5))
        exp_pool = ctx.enter_context(tc.tile_pool(name="expp", bufs=5))
        oT_pool = ctx.enter_context(tc.tile_pool(name="oT", bufs=2))
        fin_pool = ctx.enter_context(tc.tile_pool(name="fin", bufs=2))
        small = ctx.enter_context(tc.tile_pool(name="small", bufs=4))
        pp = ctx.enter_context(tc.tile_pool(name="pp", bufs=2, space="PSUM"))
        attp = ctx.enter_context(tc.tile_pool(name="attp", bufs=2, space="PSUM"))
        op = ctx.enter_context(tc.tile_pool(name="op", bufs=2, space="PSUM"))

        # ---- constants: weights, bias, identity ----
        qw_t, kvw_t, pw_t = [], [], []
        for i in range(4):
            t = consts.tile([128, C], bf16, name=f"qw{i}", tag=f"qw{i}")
            nc.sync.dma_start(t[:], qw_d.ap()[i * 128:(i + 1) * 128, :])
            qw_t.append(t)
        for i in range(4):
            t = consts.tile([128, 2 * C], bf16, name=f"kvw{i}", tag=f"kvw{i}")
            nc.sync.dma_start(t[:], kvw_d.ap()[i * 128:(i + 1) * 128, :])
            kvw_t.append(t)
        for i in range(4):
            t = consts.tile([128, C], mmdt, name=f"pw{i}", tag=f"pw{i}")
            nc.sync.dma_start(t[:], pw_d.ap()[i * 128:(i + 1) * 128, :])
            pw_t.append(t)
        ebT_t = []
        for j in range(H // 2):
            t = consts.tile([128, 2, 2 * NW], bf16, name=f"ebT{j}",
                            tag=f"ebT{j}")
            nc.sync.dma_start(t[:], ebT_d.ap()[j, :, :, :])
            ebT_t.append(t)
        pbT_t = consts.tile([128, 4], f32, name="pbT", tag="pbT")
        nc.sync.dma_start(pbT_t[:], pbT_d.ap())

        # ---- persistent double-buffered attention operands ----
        # qz[b][j]: [128, 2, SBT] block-diagonal moving q (off-diag pre-zeroed)
        # vo[b][mt]: [128, H, 2D] v columns 0:D, ones columns D:2D (pre-set)
        qz_buf = [[consts.tile([128, 2, SBT], bf16, name=f"qz{b}_{j}",
                               tag=f"qz{b}_{j}") for j in range(4)]
                  for b in range(2)]
        vo_buf = [[consts.tile([128, H, 2 * D], bf16, name=f"vo{b}_{mt}",
                               tag=f"vo{b}_{mt}") for mt in range(4)]
                  for b in range(2)]
        for b in range(2):
            for j in range(4):
                nc.gpsimd.memset(qz_buf[b][j][0:64, 1, :], 0.0)
                nc.gpsimd.memset(qz_buf[b][j][64:128, 0, :], 0.0)
            for mt in range(4):
                # ones FIRST: softmax denominators land at partition offset 0,
                # where reciprocal_approx_fast (custom DVE) is correct
                nc.gpsimd.memset(vo_buf[b][mt][:, :, 0:D], 1.0)

        # Per-sb tile state, filled by the phase closures below.
        st = {}

        def load(sb):
            ts = sb * SBT
            xt = xy_pool.tile([128, 4, SBT], bf16, name=f"xt_{sb}", tag="xt")
            nc.sync.dma_start(xt[:], xT[:, :, ts:ts + SBT])
            yt = xy_pool.tile([128, 4, SBT], bf16, name=f"yt_{sb}", tag="yt")
            nc.sync.dma_start(yt[:], yT[:, :, ts:ts + SBT])
            st[sb] = {"xt": xt, "yt": yt, "qz": qz_buf[sb % 2],
                      "vo": vo_buf[sb % 2], "kT": [None] * 4, "oT": [None] * 4}

        def proj_step(sb, i):
            """One of 12 projection steps: i in 0..3 -> q[m], 4..7 -> k[m],
            8..11 -> v[mt]."""
            s = st[sb]
            xt, yt = s["xt"], s["yt"]
            if i < 4:
                m = i
                ps = pp.tile([128, SBT], f32, name=f"qps_{sb}_{m}", tag="pp")
                for kin in range(4):
                    nc.tensor.matmul(ps[:], qw_t[kin][:, m * 128:(m + 1) * 128],
                                     xt[:, kin, :], start=(kin == 0),
                                     stop=(kin == 3))
                qz = s["qz"][m]
                sc = float(D) ** -0.5
                nc.scalar.activation(qz[0:64, 0, :], ps[0:64, :], AF.Copy,
                                     scale=sc)
                nc.scalar.activation(qz[64:128, 1, :], ps[64:128, :], AF.Copy,
                                     scale=sc)
            elif i < 8:
                m = i - 4
                ps = pp.tile([128, SBT], f32, name=f"kps_{sb}_{m}", tag="pp")
                for kin in range(4):
                    nc.tensor.matmul(ps[:], kvw_t[kin][:, m * 128:(m + 1) * 128],
                                     yt[:, kin, :], start=(kin == 0),
                                     stop=(kin == 3))
                km = k_pool.tile([128, SBT], bf16, name=f"kT_{sb}_{m}",
                                 tag=f"k{m}")
                nc.scalar.activation(km[:], ps[:], AF.Copy)
                s["kT"][m] = km
            else:
                mt = i - 8
                ps = pp.tile([128, C], f32, name=f"vps_{sb}_{mt}", tag="pp")
                for kin in range(4):
                    nc.tensor.matmul(ps[:], yt[:, kin, mt * 128:(mt + 1) * 128],
                                     kvw_t[kin][:, C:2 * C],
                                     start=(kin == 0), stop=(kin == 3))
                nc.scalar.activation(
                    s["vo"][mt][:, :, D:2 * D],
                    ps[:].rearrange("p (h d) -> p h d", h=H), AF.Copy)

        def alloc_oT(sb):
            for m in range(4):
                st[sb]["oT"][m] = oT_pool.tile([128, SBT], mmdt,
                                               name=f"oT_{sb}_{m}", tag=f"oT{m}")

        # ---- attention: 2 windows x 4 head pairs ----
        def stage_a(sb, b2, j):
            s = st[sb]
            er = esin_pool.tile([128, 2, SBT], bf16, name=f"er_{sb}_{b2}_{j}",
                                tag="er")
            aps = attp.tile([128, 2, SBT], f32, name=f"aps_{sb}_{b2}_{j}",
                            tag="attp")
            for kt in range(2):
                nc.tensor.matmul(
                    aps[:, kt, :],
                    s["kT"][j][:, b2 * NW + kt * 128:b2 * NW + (kt + 1) * 128],
                    s["qz"][j][:, :, b2 * NW:(b2 + 1) * NW],
                    start=True, stop=True)
            nc.scalar.activation(er[:], aps[:], AF.Exp)
            # multiply in exp(bias) on the (otherwise idle) Pool engine,
            # SBUF -> SBUF:  exp(z + b) = exp(z) * exp(b)
            e = exp_pool.tile([128, 2, SBT], bf16, name=f"ex_{sb}_{b2}_{j}",
                              tag="ex")
            nc.vector.tensor_mul(e[:], er[:], ebT_t[j][:])
            return e

        def stage_b(sb, b2, j, e):
            """av + normalize for one window of head pair j. ops_t rows
            0:64 = denominators (ones first in vo), 64:128 = unnormalized."""
            s = st[sb]
            ops_t = op.tile([128, SBT], f32, name=f"ops_{sb}_{b2}_{j}",
                            tag="op")
            for hh in range(2):
                h = 2 * j + hh
                for kt in range(2):
                    nc.tensor.matmul(
                        ops_t[:, hh * NW:(hh + 1) * NW],
                        s["vo"][b2 * 2 + kt][:, h, :],
                        e[:, kt, hh * NW:(hh + 1) * NW],
                        start=(kt == 0), stop=(kt == 1))
            r = small.tile([64, SBT], f32, name=f"r_{sb}_{b2}_{j}", tag="r")
            nc.vector.reciprocal_approx_fast(r[:], ops_t[0:64, :])
            for hh in range(2):
                nc.vector.tensor_mul(
                    s["oT"][j][hh * 64:(hh + 1) * 64, b2 * NW:(b2 + 1) * NW],
                    ops_t[64:128, hh * NW:(hh + 1) * NW],
                    r[:, hh * NW:(hh + 1) * NW])

        def fin(sb, next_sb=None):
            s = st.pop(sb)
            ts = sb * SBT
            fo = fin_pool.tile([128, 4, SBT], bf16, name=f"fo_{sb}", tag="fo")
            for m in range(4):
                ps = pp.tile([128, SBT], f32, name=f"fps_{sb}_{m}", tag="pp")
                for kf in range(4):
                    nc.tensor.matmul(ps[:], pw_t[kf][:, m * 128:(m + 1) * 128],
                                     s["oT"][kf][:], start=(kf == 0),
                                     stop=(kf == 3))
                nc.scalar.activation(fo[:, m, :], ps[:], AF.Identity,
                                     bias=pbT_t[:, m:m + 1], scale=1.0)
                # last deferred v-projection step fills the fin tail
                if next_sb is not None and 8 + 3 + m < 12:
                    proj_step(next_sb, 8 + 3 + m)
            nc.sync.dma_start(outT[:, :, ts:ts + SBT], fo[:])

        def attn_interleaved(sb, next_sb):
            """Attention head-pair groups of sb interleaved with projection
            steps of next_sb to keep the PE fed during exp/normalize."""
            alloc_oT(sb)
            pairs = [(b2, j) for b2 in range(2) for j in range(H // 2)]
            emitted = 0
            pending = []
            for i, (b2, j) in enumerate(pairs):
                e = stage_a(sb, b2, j)
                pending.append((b2, j, e))
                if len(pending) > 3:
                    b2p, jp, ep = pending.pop(0)
                    stage_b(sb, b2p, jp, ep)
                # interleave projection steps of the next super-batch
                want = (i + 1) * 8 // len(pairs)
                while emitted < want:
                    proj_step(next_sb, emitted)
                    emitted += 1
            for di, (b2p, jp, ep) in enumerate(pending):
                stage_b(sb, b2p, jp, ep)
                if next_sb is not None and 8 + di < 12:
                    proj_step(next_sb, 8 + di)

        def do_simple(sb, kind):
            ts = sb * SBT
            s = st[sb]
            if kind == "dmaonly":
                fo = fin_pool.tile([128, 4, SBT], f32, name=f"fo_{sb}", tag="fo")
                nc.vector.tensor_copy(fo[:], s["xt"][:].bitcast(f32))
                nc.sync.dma_start(outT[:, :, ts:ts + SBT], fo[:])
                st.pop(sb)

        def body():
            if variant != "full":
                for sb in range(nsb):
                    load(sb)
                    do_simple(sb, variant)
                return
            # steady state: attention of sb runs while projections of the
            # NEXT super-batch (wrapping into the next For_i iteration) are
            # interleaved, so there is no per-iteration pipeline bubble.
            # x/y loads are prefetched TWO super-batches ahead.
            for sb in range(nsb):
                nxt = (sb + 1) % nsb
                load((sb + 2) % nsb)
                attn_interleaved(sb, nxt)
                fin(sb, nxt)

        if variant == "full":
            # prologue: prime the pipeline for sb=0 of the first iteration
            load(0)
            load(1)
            for i in range(12):
                proj_step(0, i)
        if reps == 1:
            body()
        else:
            with tc.For_i(0, reps, 1):
                body()

    nc.compile()
    return nc


def _rel_index():
    ch = np.arange(WH)
    cw = np.arange(WW)
    yy, xx = np.meshgrid(ch, cw, indexing="ij")
    coords = np.stack([yy, xx]).reshape(2, -1)           # [2, N]
    rel = coords[:, :, None] - coords[:, None, :]        # [2, N, N]
    idx = (rel[0] + WH - 1) * (2 * WW - 1) + (rel[1] + WW - 1)
    return idx                                           # [N, N] int


def make_in_maps(x, y, q_w, kv_w, proj_w, proj_b, bias_table):
    import ml_dtypes
    bf = ml_dtypes.bfloat16
    x = np.asarray(x, dtype=np.float32)
    y = np.asarray(y, dtype=np.float32)
    q_w = np.ascontiguousarray(np.asarray(q_w, dtype=np.float32))
    kv_w = np.ascontiguousarray(np.asarray(kv_w, dtype=np.float32))
    proj_w = np.ascontiguousarray(np.asarray(proj_w, dtype=np.float32))
    proj_b = np.asarray(proj_b, dtype=np.float32)
    bias_table = np.asarray(bias_table, dtype=np.float32)

    idx = _rel_index()
    rel_bias = bias_table[idx.reshape(-1)].reshape(NW, NW, H)   # [n1, n2, h]
    biasT = rel_bias.transpose(2, 1, 0)                         # [h, k, q]
    bT = np.empty((H // 2, 2, 128, 2 * NW), np.float32)
    for j in range(H // 2):
        for kt in range(2):
            bT[j, kt, :, 0:NW] = biasT[2 * j, kt * 128:(kt + 1) * 128, :]
            bT[j, kt, :, NW:2 * NW] = biasT[2 * j + 1, kt * 128:(kt + 1) * 128, :]
    # exp(bias), laid out [j, 128 k-rows, kt, (hh,q)] for direct DMA
    ebT = np.ascontiguousarray(
        np.exp(bT).transpose(0, 2, 1, 3)).astype(bf)
    pbT = np.ascontiguousarray(proj_b.reshape(4, 128).T)        # [128, 4]

    in_maps = []
    for c in range(N_CORES):
        xc = x[c * BC:(c + 1) * BC].reshape(T, C)
        yc = y[c * BC:(c + 1) * BC].reshape(T, C)
        # packed feature-major layout: [128, 4, T]; [p, kin, t] = xT[kin*128+p, t]
        xp = np.ascontiguousarray(xc.T.reshape(4, 128, T).transpose(1, 0, 2))
        yp = np.ascontiguousarray(yc.T.reshape(4, 128, T).transpose(1, 0, 2))
        in_maps.append({
            "xT": xp.astype(bf), "yT": yp.astype(bf),
            "qw": q_w.astype(bf), "kvw": kv_w.astype(bf),
            "pw": proj_w, "pbT": pbT, "ebT": ebT,
        })
    return in_maps


def unpack_out(o):
    """[128, 4, T] packed outT -> [BC, NW, C] per-core output."""
    return np.asarray(o).transpose(1, 0, 2).reshape(C, T).T.reshape(BC, NW, C)


_CACHE = {}


def kernel(x, y, q_w, kv_w, proj_w, proj_b, bias_table):
    import sys
    if _TRN_REPO not in sys.path:
        sys.path.insert(0, _TRN_REPO)
    from concourse.bass_utils import run_bass_kernel_spmd

    if "nc" not in _CACHE:
        _CACHE["nc"] = build_module()
    nc = _CACHE["nc"]

    in_maps = make_in_maps(x, y, q_w, kv_w, proj_w, proj_b, bias_table)
    res = run_bass_kernel_spmd(nc, in_maps, core_ids=list(range(N_CORES)))
    outs = [unpack_out(res.results[c]["outT"]) for c in range(N_CORES)]
    return np.ascontiguousarray(np.concatenate(outs, axis=0), dtype=np.float32)


# revision 56
# speedup vs baseline: 1.0021x; 1.0021x over previous
"""Windowed cross-attention (sparse_attention) on Trainium2.

Data-parallel over the batch axis across 8 NeuronCores; each core processes
16 windows (4096 tokens) of the B=128 batch. Projections run in float32r
(full PE rate); the attention core (q*k logits, exp, attn*v) runs in bf16
inputs with fp32 PSUM accumulation. Host pre-transposes x/y to feature-major
layout and pre-bakes the relative-position bias per head pair so the device
program is pure matmul + softmax with no on-device transposes or gathers:

  qT = (q_w.T @ xT) * scale            (feature-major, written zero-padded)
  kT = kv_w[:, :C].T @ yT              (feature-major)
  v  = yT.T-tiles @ kv_w[:, C:]        (token-major, + 64 ones columns)
  attnT[k, (kt,hh,q)] = I.T @ biasT  +  kT.T @ qz   (one 512-col matmul each)
  expT = exp(attnT)                    (one 1024-col ACT op per head pair)
  ops = [v | 1s].T @ expT   -> rows 0:64 = unnormalized outT,
                               rows 64:128 = softmax denominator (x64)
  outT = ops[0:64] * reciprocal(ops[64:128])  (DVE only, no broadcasts)
  finT = proj_w.T-slices @ outT + proj_b      (bias via ACT Identity)

Head pairs (2j, 2j+1) occupy partition halves 0:64 / 64:128 of feature tile
j. The moving q tensor is stored block-diagonal ([128, 2, SBT] with the
off-diagonal blocks zero, pre-zeroed once in persistent buffers) so a single
128-contraction matmul computes both heads' logits into disjoint column
halves. x/y/out use a packed [128, 4, T] layout (one DMA each per
super-batch); the ones columns live in persistent v buffers written once at
module start. The projection work of super-batch sb+1 is interleaved at
head-pair granularity into the attention phase of super-batch sb so the
tensor engine always has independent work while ACT (exp) and DVE
(normalize) catch up.
"""

import numpy as np

_TRN_REPO = "/opt/trn_rl_repo"
N_CORES = 8
B, NW, C = 128, 256, 512        # full batch, window tokens, channels
H, D = 8, 64                    # heads, head dim
WH = WW = 16
BC = B // N_CORES               # windows per core
T = BC * NW                     # tokens per core
NSB_FULL = 8                    # super-batches (2 windows each) per core
SBT = T // NSB_FULL             # tokens per super-batch


def build_module(reps=1, mm="float32r", nsb=NSB_FULL, variant="full"):
    """Build + compile the per-core Bass module (SPMD; same program all cores)."""
    import sys
    if _TRN_REPO not in sys.path:
        sys.path.insert(0, _TRN_REPO)
    from contextlib import ExitStack

    import concourse.bacc as bacc
    import concourse.tile as tile
    from concourse import mybir

    f32 = mybir.dt.float32
    bf16 = mybir.dt.bfloat16
    mmdt = getattr(mybir.dt, mm)
    AF = mybir.ActivationFunctionType

    nc = bacc.Bacc("TRN2", debug=False, enable_asserts=False, num_devices=N_CORES)
    xyT_d = nc.dram_tensor("xyT", [128, 2, 4, T], bf16, kind="ExternalInput")
    qw_d = nc.dram_tensor("qw", [C, C], bf16, kind="ExternalInput")
    kvw_d = nc.dram_tensor("kvw", [C, 2 * C], bf16, kind="ExternalInput")
    pw_d = nc.dram_tensor("pw", [C, C], mmdt, kind="ExternalInput")
    pbT_d = nc.dram_tensor("pbT", [128, 4], f32, kind="ExternalInput")
    # paired exp-bias: ebT[j, :, kt] = exp(bias)[128 k-rows,
    #                                           256q(head 2j) | 256q(head 2j+1)]
    ebT_d = nc.dram_tensor("ebT", [H // 2, 128, 2, 2 * NW], bf16,
                           kind="ExternalInput")
    outT_d = nc.dram_tensor("outT", [128, 4, T], bf16, kind="ExternalOutput")

    xyT, outT = xyT_d.ap(), outT_d.ap()

    with tile.TileContext(nc) as tc, ExitStack() as ctx:
        ctx.enter_context(nc.allow_low_precision(
            reason="attention core in bf16; projections f32r; fp32 accumulation"))
        consts = ctx.enter_context(tc.tile_pool(name="consts", bufs=1))
        xy_pool = ctx.enter_context(tc.tile_pool(name="xy", bufs=4))
        k_pool = ctx.enter_context(tc.tile_pool(name="kp", bufs=2))
        esin_pool = ctx.enter_context(tc.tile_pool(name="esin", bufs=5))
        exp_pool = ctx.enter_context(tc.tile_pool(name="expp", bufs=5))
        oT_pool = ctx.enter_context(tc.tile_pool(name="oT", bufs=2))
        fin_pool = ctx.enter_context(tc.tile_pool(name="fin", bufs=2))
        small = ctx.enter_context(tc.tile_pool(name="small", bufs=4))
        pp = ctx.enter_context(tc.tile_pool(name="pp", bufs=2, space="PSUM"))
        attp = ctx.enter_context(tc.tile_pool(name="attp", bufs=2, space="PSUM"))
        op = ctx.enter_context(tc.tile_pool(name="op", bufs=2, space="PSUM"))

        # ---- constants: weights, bias, identity ----
        qw_t, kvw_t, pw_t = [], [], []
        for i in range(4):
            t = consts.tile([128, C], bf16, name=f"qw{i}", tag=f"qw{i}")
            nc.sync.dma_start(t[:], qw_d.ap()[i * 128:(i + 1) * 128, :])
            qw_t.append(t)
        for i in range(4):
            t = consts.tile([128, 2 * C], bf16, name=f"kvw{i}", tag=f"kvw{i}")
            nc.sync.dma_start(t[:], kvw_d.ap()[i * 128:(i + 1) * 128, :])
            kvw_t.append(t)
        for i in range(4):
            t = consts.tile([128, C], mmdt, name=f"pw{i}", tag=f"pw{i}")
            nc.sync.dma_start(t[:], pw_d.ap()[i * 128:(i + 1) * 128, :])
            pw_t.append(t)
        ebT_t = []
        for j in range(H // 2):
            t = consts.tile([128, 2, 2 * NW], bf16, name=f"ebT{j}",
                            tag=f"ebT{j}")
            nc.sync.dma_start(t[:], ebT_d.ap()[j, :, :, :])
            ebT_t.append(t)
        pbT_t = consts.tile([128, 4], f32, name="pbT", tag="pbT")
        nc.sync.dma_start(pbT_t[:], pbT_d.ap())

        # ---- persistent double-buffered attention operands ----
        # qz[b][j]: [128, 2, SBT] block-diagonal moving q (off-diag pre-zeroed)
        # vo[b][mt]: [128, H, 2D] v columns 0:D, ones columns D:2D (pre-set)
        qz_buf = [[consts.tile([128, 2, SBT], bf16, name=f"qz{b}_{j}",
                               tag=f"qz{b}_{j}") for j in range(4)]
                  for b in range(2)]
        vo_buf = [[consts.tile([128, H, 2 * D], bf16, name=f"vo{b}_{mt}",
                               tag=f"vo{b}_{mt}") for mt in range(4)]
                  for b in range(2)]
        for b in range(2):
            for j in range(4):
                nc.gpsimd.memset(qz_buf[b][j][0:64, 1, :], 0.0)
                nc.gpsimd.memset(qz_buf[b][j][64:128, 0, :], 0.0)
            for mt in range(4):
                # ones FIRST: softmax denominators land at partition offset 0,
                # where reciprocal_approx_fast (custom DVE) is correct
                nc.gpsimd.memset(vo_buf[b][mt][:, :, 0:D], 1.0)

        # Per-sb tile state, filled by the phase closures below.
        st = {}

        def load(sb):
            ts = sb * SBT
            xyt = xy_pool.tile([128, 2, 4, SBT], bf16, name=f"xyt_{sb}",
                               tag="xyt")
            nc.sync.dma_start(xyt[:], xyT[:, :, :, ts:ts + SBT])
            st[sb] = {"xt": xyt[:, 0, :, :], "yt": xyt[:, 1, :, :],
                      "qz": qz_buf[sb % 2], "vo": vo_buf[sb % 2],
                      "kT": [None] * 4, "oT": [None] * 4}

        def proj_step(sb, i):
            """One of 12 projection steps: i in 0..3 -> q[m], 4..7 -> k[m],
            8..11 -> v[mt]."""
            s = st[sb]
            xt, yt = s["xt"], s["yt"]
            if i < 4:
                m = i
                ps = pp.tile([128, SBT], f32, name=f"qps_{sb}_{m}", tag="pp")
                for kin in range(4):
                    nc.tensor.matmul(ps[:], qw_t[kin][:, m * 128:(m + 1) * 128],
                                     xt[:, kin, :], start=(kin == 0),
                                     stop=(kin == 3))
                qz = s["qz"][m]
                sc = float(D) ** -0.5
                nc.scalar.activation(qz[0:64, 0, :], ps[0:64, :], AF.Copy,
                                     scale=sc)
                nc.scalar.activation(qz[64:128, 1, :], ps[64:128, :], AF.Copy,
                                     scale=sc)
            elif i < 8:
                m = i - 4
                ps = pp.tile([128, SBT], f32, name=f"kps_{sb}_{m}", tag="pp")
                for kin in range(4):
                    nc.tensor.matmul(ps[:], kvw_t[kin][:, m * 128:(m + 1) * 128],
                                     yt[:, kin, :], start=(kin == 0),
                                     stop=(kin == 3))
                km = k_pool.tile([128, SBT], bf16, name=f"kT_{sb}_{m}",
                                 tag=f"k{m}")
                nc.scalar.activation(km[:], ps[:], AF.Copy)
                s["kT"][m] = km
            else:
                mt = i - 8
                ps = pp.tile([128, C], f32, name=f"vps_{sb}_{mt}", tag="pp")
                for kin in range(4):
                    nc.tensor.matmul(ps[:], yt[:, kin, mt * 128:(mt + 1) * 128],
                                     kvw_t[kin][:, C:2 * C],
                                     start=(kin == 0), stop=(kin == 3))
                nc.scalar.activation(
                    s["vo"][mt][:, :, D:2 * D],
                    ps[:].rearrange("p (h d) -> p h d", h=H), AF.Copy)

        def alloc_oT(sb):
            for m in range(4):
                st[sb]["oT"][m] = oT_pool.tile([128, SBT], mmdt,
                                               name=f"oT_{sb}_{m}", tag=f"oT{m}")

        # ---- attention: 2 windows x 4 head pairs ----
        def stage_a(sb, b2, j):
            s = st[sb]
            er = esin_pool.tile([128, 2, SBT], bf16, name=f"er_{sb}_{b2}_{j}",
                                tag="er")
            aps = attp.tile([128, 2, SBT], f32, name=f"aps_{sb}_{b2}_{j}",
                            tag="attp")
            for kt in range(2):
                nc.tensor.matmul(
                    aps[:, kt, :],
                    s["kT"][j][:, b2 * NW + kt * 128:b2 * NW + (kt + 1) * 128],
                    s["qz"][j][:, :, b2 * NW:(b2 + 1) * NW],
                    start=True, stop=True)
            nc.scalar.activation(er[:], aps[:], AF.Exp)
            # multiply in exp(bias) on the (otherwise idle) Pool engine,
            # SBUF -> SBUF:  exp(z + b) = exp(z) * exp(b)
            e = exp_pool.tile([128, 2, SBT], bf16, name=f"ex_{sb}_{b2}_{j}",
                              tag="ex")
            nc.vector.tensor_mul(e[:], er[:], ebT_t[j][:])
            return e

        def stage_b(sb, b2, j, e):
            """av + normalize for one window of head pair j. ops_t rows
            0:64 = denominators (ones first in vo), 64:128 = unnormalized."""
            s = st[sb]
            ops_t = op.tile([128, SBT], f32, name=f"ops_{sb}_{b2}_{j}",
                            tag="op")
            for hh in range(2):
                h = 2 * j + hh
                for kt in range(2):
                    nc.tensor.matmul(
                        ops_t[:, hh * NW:(hh + 1) * NW],
                        s["vo"][b2 * 2 + kt][:, h, :],
                        e[:, kt, hh * NW:(hh + 1) * NW],
                        start=(kt == 0), stop=(kt == 1))
            r = small.tile([64, SBT], f32, name=f"r_{sb}_{b2}_{j}", tag="r")
            nc.vector.reciprocal_approx_fast(r[:], ops_t[0:64, :])
            for hh in range(2):
                nc.vector.tensor_mul(
                    s["oT"][j][hh * 64:(hh + 1) * 64, b2 * NW:(b2 + 1) * NW],
                    ops_t[64:128, hh * NW:(hh + 1) * NW],
                    r[:, hh * NW:(hh + 1) * NW])

        def fin(sb, next_sb=None):
            s = st.pop(sb)
            ts = sb * SBT
            fo = fin_pool.tile([128, 4, SBT], bf16, name=f"fo_{sb}", tag="fo")
            for m in range(4):
                ps = pp.tile([128, SBT], f32, name=f"fps_{sb}_{m}", tag="pp")
                for kf in range(4):
                    nc.tensor.matmul(ps[:], pw_t[kf][:, m * 128:(m + 1) * 128],
                                     s["oT"][kf][:], start=(kf == 0),
                                     stop=(kf == 3))
                nc.scalar.activation(fo[:, m, :], ps[:], AF.Identity,
                                     bias=pbT_t[:, m:m + 1], scale=1.0)
                # last deferred v-projection step fills the fin tail
                if next_sb is not None and 8 + 3 + m < 12:
                    proj_step(next_sb, 8 + 3 + m)
            nc.sync.dma_start(outT[:, :, ts:ts + SBT], fo[:])

        def attn_interleaved(sb, next_sb):
            """Attention head-pair groups of sb interleaved with projection
            steps of next_sb to keep the PE fed during exp/normalize."""
            alloc_oT(sb)
            pairs = [(b2, j) for b2 in range(2) for j in range(H // 2)]
            emitted = 0
            pending = []
            for i, (b2, j) in enumerate(pairs):
                e = stage_a(sb, b2, j)
                pending.append((b2, j, e))
                if len(pending) > 3:
                    b2p, jp, ep = pending.pop(0)
                    stage_b(sb, b2p, jp, ep)
                # interleave projection steps of the next super-batch
                want = (i + 1) * 8 // len(pairs)
                while emitted < want:
                    proj_step(next_sb, emitted)
                    emitted += 1
            for di, (b2p, jp, ep) in enumerate(pending):
                stage_b(sb, b2p, jp, ep)
                if next_sb is not None and 8 + di < 12:
                    proj_step(next_sb, 8 + di)

        def do_simple(sb, kind):
            ts = sb * SBT
            s = st[sb]
            if kind == "dmaonly":
                fo = fin_pool.tile([128, 4, SBT], f32, name=f"fo_{sb}", tag="fo")
                nc.vector.tensor_copy(fo[:], s["xt"][:].bitcast(f32))
                nc.sync.dma_start(outT[:, :, ts:ts + SBT], fo[:])
                st.pop(sb)

        def body():
            if variant != "full":
                for sb in range(nsb):
                    load(sb)
                    do_simple(sb, variant)
                return
            # steady state: attention of sb runs while projections of the
            # NEXT super-batch (wrapping into the next For_i iteration) are
            # interleaved, so there is no per-iteration pipeline bubble.
            # x/y loads are prefetched TWO super-batches ahead.
            for sb in range(nsb):
                nxt = (sb + 1) % nsb
                load((sb + 2) % nsb)
                attn_interleaved(sb, nxt)
                fin(sb, nxt)

        if variant == "full":
            # prologue: prime the pipeline for sb=0 of the first iteration
            load(0)
            load(1)
            for i in range(12):
                proj_step(0, i)
        if reps == 1:
            body()
        else:
            with tc.For_i(0, reps, 1):
                body()

    nc.compile()
    return nc


def _rel_index():
    ch = np.arange(WH)
    cw = np.arange(WW)
    yy, xx = np.meshgrid(ch, cw, indexing="ij")
    coords = np.stack([yy, xx]).reshape(2, -1)           # [2, N]
    rel = coords[:, :, None] - coords[:, None, :]        # [2, N, N]
    idx = (rel[0] + WH - 1) * (2 * WW - 1) + (rel[1] + WW - 1)
    return idx                                           # [N, N] int


def make_in_maps(x, y, q_w, kv_w, proj_w, proj_b, bias_table):
    import ml_dtypes
    bf = ml_dtypes.bfloat16
    x = np.asarray(x, dtype=np.float32)
    y = np.asarray(y, dtype=np.float32)
    q_w = np.ascontiguousarray(np.asarray(q_w, dtype=np.float32))
    kv_w = np.ascontiguousarray(np.asarray(kv_w, dtype=np.float32))
    proj_w = np.ascontiguousarray(np.asarray(proj_w, dtype=np.float32))
    proj_b = np.asarray(proj_b, dtype=np.float32)
    bias_table = np.asarray(bias_table, dtype=np.float32)

    idx = _rel_index()
    rel_bias = bias_table[idx.reshape(-1)].reshape(NW, NW, H)   # [n1, n2, h]
    biasT = rel_bias.transpose(2, 1, 0)                         # [h, k, q]
    bT = np.empty((H // 2, 2, 128, 2 * NW), np.float32)
    for j in range(H // 2):
        for kt in range(2):
            bT[j, kt, :, 0:NW] = biasT[2 * j, kt * 128:(kt + 1) * 128, :]
            bT[j, kt, :, NW:2 * NW] = biasT[2 * j + 1, kt * 128:(kt + 1) * 128, :]
    # exp(bias), laid out [j, 128 k-rows, kt, (hh,q)] for direct DMA
    ebT = np.ascontiguousarray(
        np.exp(bT).transpose(0, 2, 1, 3)).astype(bf)
    pbT = np.ascontiguousarray(proj_b.reshape(4, 128).T)        # [128, 4]

    in_maps = []
    for c in range(N_CORES):
        xc = x[c * BC:(c + 1) * BC].reshape(T, C)
        yc = y[c * BC:(c + 1) * BC].reshape(T, C)
        # packed feature-major layout: [128, 2, 4, T];
        # [p, 0, kin, t] = xT[kin*128+p, t], [p, 1, kin, t] = yT[...]
        xp = xc.T.reshape(4, 128, T).transpose(1, 0, 2)
        yp = yc.T.reshape(4, 128, T).transpose(1, 0, 2)
        xyp = np.ascontiguousarray(np.stack([xp, yp], axis=1)).astype(bf)
        in_maps.append({
            "xyT": xyp,
            "qw": q_w.astype(bf), "kvw": kv_w.astype(bf),
            "pw": proj_w, "pbT": pbT, "ebT": ebT,
        })
    return in_maps


def unpack_out(o):
    """[128, 4, T] packed outT -> [BC, NW, C] per-core output."""
    return np.asarray(o).transpose(1, 0, 2).reshape(C, T).T.reshape(BC, NW, C)


_CACHE = {}


def kernel(x, y, q_w, kv_w, proj_w, proj_b, bias_table):
    import sys
    if _TRN_REPO not in sys.path:
        sys.path.insert(0, _TRN_REPO)
    from concourse.bass_utils import run_bass_kernel_spmd

    if "nc" not in _CACHE:
        _CACHE["nc"] = build_module()
    nc = _CACHE["nc"]

    in_maps = make_in_maps(x, y, q_w, kv_w, proj_w, proj_b, bias_table)
    res = run_bass_kernel_spmd(nc, in_maps, core_ids=list(range(N_CORES)))
    outs = [unpack_out(res.results[c]["outT"]) for c in range(N_CORES)]
    return np.ascontiguousarray(np.concatenate(outs, axis=0), dtype=np.float32)


# revision 58
# speedup vs baseline: 1.0306x; 1.0285x over previous
"""Windowed cross-attention (sparse_attention) on Trainium2.

Data-parallel over the batch axis across 8 NeuronCores; each core processes
16 windows (4096 tokens) of the B=128 batch. Projections run in float32r
(full PE rate); the attention core (q*k logits, exp, attn*v) runs in bf16
inputs with fp32 PSUM accumulation. Host pre-transposes x/y to feature-major
layout and pre-bakes the relative-position bias per head pair so the device
program is pure matmul + softmax with no on-device transposes or gathers:

  qT = (q_w.T @ xT) * scale            (feature-major, written zero-padded)
  kT = kv_w[:, :C].T @ yT              (feature-major)
  v  = yT.T-tiles @ kv_w[:, C:]        (token-major, + 64 ones columns)
  attnT[k, (kt,hh,q)] = I.T @ biasT  +  kT.T @ qz   (one 512-col matmul each)
  expT = exp(attnT)                    (one 1024-col ACT op per head pair)
  ops = [v | 1s].T @ expT   -> rows 0:64 = unnormalized outT,
                               rows 64:128 = softmax denominator (x64)
  outT = ops[0:64] * reciprocal(ops[64:128])  (DVE only, no broadcasts)
  finT = proj_w.T-slices @ outT + proj_b      (bias via ACT Identity)

Head pairs (2j, 2j+1) occupy partition halves 0:64 / 64:128 of feature tile
j. The moving q tensor is stored block-diagonal ([128, 2, SBT] with the
off-diagonal blocks zero, pre-zeroed once in persistent buffers) so a single
128-contraction matmul computes both heads' logits into disjoint column
halves. x/y/out use a packed [128, 4, T] layout (one DMA each per
super-batch); the ones columns live in persistent v buffers written once at
module start. The projection work of super-batch sb+1 is interleaved at
head-pair granularity into the attention phase of super-batch sb so the
tensor engine always has independent work while ACT (exp) and DVE
(normalize) catch up.
"""

import numpy as np

_TRN_REPO = "/opt/trn_rl_repo"
N_CORES = 8
B, NW, C = 128, 256, 512        # full batch, window tokens, channels
H, D = 8, 64                    # heads, head dim
WH = WW = 16
BC = B // N_CORES               # windows per core
T = BC * NW                     # tokens per core
NSB_FULL = 8                    # super-batches (2 windows each) per core
SBT = T // NSB_FULL             # tokens per super-batch


def build_module(reps=1, mm="float32r", nsb=NSB_FULL, variant="full"):
    """Build + compile the per-core Bass module (SPMD; same program all cores)."""
    import sys
    if _TRN_REPO not in sys.path:
        sys.path.insert(0, _TRN_REPO)
    from contextlib import ExitStack

    import concourse.bacc as bacc
    import concourse.tile as tile
    from concourse import mybir

    f32 = mybir.dt.float32
    bf16 = mybir.dt.bfloat16
    mmdt = getattr(mybir.dt, mm)
    AF = mybir.ActivationFunctionType

    nc = bacc.Bacc("TRN2", debug=False, enable_asserts=False, num_devices=N_CORES)
    xT_d = nc.dram_tensor("xT", [128, 4, T], bf16, kind="ExternalInput")
    yT_d = nc.dram_tensor("yT", [128, 4, T], bf16, kind="ExternalInput")
    qw_d = nc.dram_tensor("qw", [C, C], bf16, kind="ExternalInput")
    kvw_d = nc.dram_tensor("kvw", [C, 2 * C], bf16, kind="ExternalInput")
    pw_d = nc.dram_tensor("pw", [C, C], mmdt, kind="ExternalInput")
    pbT_d = nc.dram_tensor("pbT", [128, 4], f32, kind="ExternalInput")
    # paired exp-bias: ebT[j, :, kt] = exp(bias)[128 k-rows,
    #                                           256q(head 2j) | 256q(head 2j+1)]
    ebT_d = nc.dram_tensor("ebT", [H // 2, 128, 2, 2 * NW], bf16,
                           kind="ExternalInput")
    outT_d = nc.dram_tensor("outT", [128, 4, T], bf16, kind="ExternalOutput")

    xT, yT, outT = xT_d.ap(), yT_d.ap(), outT_d.ap()

    with tile.TileContext(nc) as tc, ExitStack() as ctx:
        ctx.enter_context(nc.allow_low_precision(
            reason="attention core in bf16; projections f32r; fp32 accumulation"))
        consts = ctx.enter_context(tc.tile_pool(name="consts", bufs=1))
        xy_pool = ctx.enter_context(tc.tile_pool(name="xy", bufs=4))
        k_pool = ctx.enter_context(tc.tile_pool(name="kp", bufs=2))
        esin_pool = ctx.enter_context(tc.tile_pool(name="esin", bufs=5))
        exp_pool = ctx.enter_context(tc.tile_pool(name="expp", bufs=5))
        oT_pool = ctx.enter_context(tc.tile_pool(name="oT", bufs=2))
        fin_pool = ctx.enter_context(tc.tile_pool(name="fin", bufs=2))
        small = ctx.enter_context(tc.tile_pool(name="small", bufs=4))
        pp = ctx.enter_context(tc.tile_pool(name="pp", bufs=2, space="PSUM"))
        attp = ctx.enter_context(tc.tile_pool(name="attp", bufs=2, space="PSUM"))
        op = ctx.enter_context(tc.tile_pool(name="op", bufs=2, space="PSUM"))

        # ---- constants: weights, bias, identity ----
        qw_t, kvw_t, pw_t = [], [], []
        for i in range(4):
            t = consts.tile([128, C], bf16, name=f"qw{i}", tag=f"qw{i}")
            nc.sync.dma_start(t[:], qw_d.ap()[i * 128:(i + 1) * 128, :])
            qw_t.append(t)
        for i in range(4):
            t = consts.tile([128, 2 * C], bf16, name=f"kvw{i}", tag=f"kvw{i}")
            nc.sync.dma_start(t[:], kvw_d.ap()[i * 128:(i + 1) * 128, :])
            kvw_t.append(t)
        for i in range(4):
            t = consts.tile([128, C], mmdt, name=f"pw{i}", tag=f"pw{i}")
            nc.sync.dma_start(t[:], pw_d.ap()[i * 128:(i + 1) * 128, :])
            pw_t.append(t)
        ebT_t = []
        for j in range(H // 2):
            t = consts.tile([128, 2, 2 * NW], bf16, name=f"ebT{j}",
                            tag=f"ebT{j}")
            nc.sync.dma_start(t[:], ebT_d.ap()[j, :, :, :])
            ebT_t.append(t)
        pbT_t = consts.tile([128, 4], f32, name="pbT", tag="pbT")
        nc.sync.dma_start(pbT_t[:], pbT_d.ap())

        # ---- persistent double-buffered attention operands ----
        # qz[b][j]: [128, 2, SBT] block-diagonal moving q (off-diag pre-zeroed)
        # vo[b][mt]: [128, H, 2D] v columns 0:D, ones columns D:2D (pre-set)
        qz_buf = [[consts.tile([128, 2, SBT], bf16, name=f"qz{b}_{j}",
                               tag=f"qz{b}_{j}") for j in range(4)]
                  for b in range(2)]
        vo_buf = [[consts.tile([128, H, 2 * D], bf16, name=f"vo{b}_{mt}",
                               tag=f"vo{b}_{mt}") for mt in range(4)]
                  for b in range(2)]
        for b in range(2):
            for j in range(4):
                nc.gpsimd.memset(qz_buf[b][j][0:64, 1, :], 0.0)
                nc.gpsimd.memset(qz_buf[b][j][64:128, 0, :], 0.0)
            for mt in range(4):
                # ones FIRST: softmax denominators land at partition offset 0,
                # where reciprocal_approx_fast (custom DVE) is correct
                nc.gpsimd.memset(vo_buf[b][mt][:, :, 0:D], 1.0)

        # Per-sb tile state, filled by the phase closures below.
        st = {}

        def load(sb):
            ts = sb * SBT
            xt = xy_pool.tile([128, 4, SBT], bf16, name=f"xt_{sb}", tag="xt")
            nc.sync.dma_start(xt[:], xT[:, :, ts:ts + SBT])
            yt = xy_pool.tile([128, 4, SBT], bf16, name=f"yt_{sb}", tag="yt")
            nc.sync.dma_start(yt[:], yT[:, :, ts:ts + SBT])
            st[sb] = {"xt": xt, "yt": yt, "qz": qz_buf[sb % 2],
                      "vo": vo_buf[sb % 2], "kT": [None] * 4, "oT": [None] * 4}

        def proj_step(sb, i):
            """One of 12 projection steps: i in 0..3 -> q[m], 4..7 -> k[m],
            8..11 -> v[mt]."""
            s = st[sb]
            xt, yt = s["xt"], s["yt"]
            if i < 4:
                m = i
                ps = pp.tile([128, SBT], f32, name=f"qps_{sb}_{m}", tag="pp")
                for kin in range(4):
                    nc.tensor.matmul(ps[:], qw_t[kin][:, m * 128:(m + 1) * 128],
                                     xt[:, kin, :], start=(kin == 0),
                                     stop=(kin == 3))
                qz = s["qz"][m]
                sc = float(D) ** -0.5
                nc.scalar.activation(qz[0:64, 0, :], ps[0:64, :], AF.Copy,
                                     scale=sc)
                nc.scalar.activation(qz[64:128, 1, :], ps[64:128, :], AF.Copy,
                                     scale=sc)
            elif i < 8:
                m = i - 4
                ps = pp.tile([128, SBT], f32, name=f"kps_{sb}_{m}", tag="pp")
                for kin in range(4):
                    nc.tensor.matmul(ps[:], kvw_t[kin][:, m * 128:(m + 1) * 128],
                                     yt[:, kin, :], start=(kin == 0),
                                     stop=(kin == 3))
                km = k_pool.tile([128, SBT], bf16, name=f"kT_{sb}_{m}",
                                 tag=f"k{m}")
                nc.scalar.activation(km[:], ps[:], AF.Copy)
                s["kT"][m] = km
            else:
                mt = i - 8
                ps = pp.tile([128, C], f32, name=f"vps_{sb}_{mt}", tag="pp")
                for kin in range(4):
                    nc.tensor.matmul(ps[:], yt[:, kin, mt * 128:(mt + 1) * 128],
                                     kvw_t[kin][:, C:2 * C],
                                     start=(kin == 0), stop=(kin == 3))
                nc.scalar.activation(
                    s["vo"][mt][:, :, D:2 * D],
                    ps[:].rearrange("p (h d) -> p h d", h=H), AF.Copy)

        def alloc_oT(sb):
            for m in range(4):
                st[sb]["oT"][m] = oT_pool.tile([128, SBT], mmdt,
                                               name=f"oT_{sb}_{m}", tag=f"oT{m}")

        # ---- attention: 2 windows x 4 head pairs ----
        def stage_a(sb, b2, j):
            s = st[sb]
            er = esin_pool.tile([128, 2, SBT], bf16, name=f"er_{sb}_{b2}_{j}",
                                tag="er")
            aps = attp.tile([128, 2, SBT], f32, name=f"aps_{sb}_{b2}_{j}",
                            tag="attp")
            for kt in range(2):
                nc.tensor.matmul(
                    aps[:, kt, :],
                    s["kT"][j][:, b2 * NW + kt * 128:b2 * NW + (kt + 1) * 128],
                    s["qz"][j][:, :, b2 * NW:(b2 + 1) * NW],
                    start=True, stop=True)
            nc.scalar.activation(er[:], aps[:], AF.Exp)
            # multiply in exp(bias) on the (otherwise idle) Pool engine,
            # SBUF -> SBUF:  exp(z + b) = exp(z) * exp(b)
            e = exp_pool.tile([128, 2, SBT], bf16, name=f"ex_{sb}_{b2}_{j}",
                              tag="ex")
            nc.vector.tensor_mul(e[:], er[:], ebT_t[j][:])
            return e

        def stage_b(sb, b2, j, e):
            """av + normalize for one window of head pair j. ops_t rows
            0:64 = denominators (ones first in vo), 64:128 = unnormalized."""
            s = st[sb]
            ops_t = op.tile([128, SBT], f32, name=f"ops_{sb}_{b2}_{j}",
                            tag="op")
            for hh in range(2):
                h = 2 * j + hh
                for kt in range(2):
                    nc.tensor.matmul(
                        ops_t[:, hh * NW:(hh + 1) * NW],
                        s["vo"][b2 * 2 + kt][:, h, :],
                        e[:, kt, hh * NW:(hh + 1) * NW],
                        start=(kt == 0), stop=(kt == 1))
            r = small.tile([64, SBT], f32, name=f"r_{sb}_{b2}_{j}", tag="r")
            nc.vector.reciprocal_approx_fast(r[:], ops_t[0:64, :])
            for hh in range(2):
                nc.vector.tensor_mul(
                    s["oT"][j][hh * 64:(hh + 1) * 64, b2 * NW:(b2 + 1) * NW],
                    ops_t[64:128, hh * NW:(hh + 1) * NW],
                    r[:, hh * NW:(hh + 1) * NW])

        def fin(sb, next_sb=None):
            s = st.pop(sb)
            ts = sb * SBT
            fo = fin_pool.tile([128, 4, SBT], bf16, name=f"fo_{sb}", tag="fo")
            for m in range(4):
                ps = op.tile([128, SBT], f32, name=f"fps_{sb}_{m}", tag="op")
                for kf in range(4):
                    nc.tensor.matmul(ps[:], pw_t[kf][:, m * 128:(m + 1) * 128],
                                     s["oT"][kf][:], start=(kf == 0),
                                     stop=(kf == 3))
                nc.scalar.activation(fo[:, m, :], ps[:], AF.Identity,
                                     bias=pbT_t[:, m:m + 1], scale=1.0)
                # last deferred v-projection step fills the fin tail
                if next_sb is not None and 8 + 3 + m < 12:
                    proj_step(next_sb, 8 + 3 + m)
            nc.sync.dma_start(outT[:, :, ts:ts + SBT], fo[:])

        def attn_interleaved(sb, next_sb):
            """Attention head-pair groups of sb interleaved with projection
            steps of next_sb to keep the PE fed during exp/normalize."""
            alloc_oT(sb)
            pairs = [(b2, j) for b2 in range(2) for j in range(H // 2)]
            emitted = 0
            pending = []
            for i, (b2, j) in enumerate(pairs):
                e = stage_a(sb, b2, j)
                pending.append((b2, j, e))
                if len(pending) > 3:
                    b2p, jp, ep = pending.pop(0)
                    stage_b(sb, b2p, jp, ep)
                # interleave projection steps of the next super-batch
                want = (i + 1) * 8 // len(pairs)
                while emitted < want:
                    proj_step(next_sb, emitted)
                    emitted += 1
            for di, (b2p, jp, ep) in enumerate(pending):
                stage_b(sb, b2p, jp, ep)
                if next_sb is not None and 8 + di < 12:
                    proj_step(next_sb, 8 + di)

        def do_simple(sb, kind):
            ts = sb * SBT
            s = st[sb]
            if kind == "dmaonly":
                fo = fin_pool.tile([128, 4, SBT], f32, name=f"fo_{sb}", tag="fo")
                nc.vector.tensor_copy(fo[:], s["xt"][:].bitcast(f32))
                nc.sync.dma_start(outT[:, :, ts:ts + SBT], fo[:])
                st.pop(sb)

        def body():
            if variant != "full":
                for sb in range(nsb):
                    load(sb)
                    do_simple(sb, variant)
                return
            # steady state: attention of sb runs while projections of the
            # NEXT super-batch (wrapping into the next For_i iteration) are
            # interleaved, so there is no per-iteration pipeline bubble.
            # x/y loads are prefetched TWO super-batches ahead.
            for sb in range(nsb):
                nxt = (sb + 1) % nsb
                load((sb + 2) % nsb)
                attn_interleaved(sb, nxt)
                fin(sb, nxt)

        if variant == "full":
            # prologue: prime the pipeline for sb=0 of the first iteration
            load(0)
            load(1)
            for i in range(12):
                proj_step(0, i)
        if reps == 1:
            body()
        else:
            with tc.For_i(0, reps, 1):
                body()

    nc.compile()
    return nc


def _rel_index():
    ch = np.arange(WH)
    cw = np.arange(WW)
    yy, xx = np.meshgrid(ch, cw, indexing="ij")
    coords = np.stack([yy, xx]).reshape(2, -1)           # [2, N]
    rel = coords[:, :, None] - coords[:, None, :]        # [2, N, N]
    idx = (rel[0] + WH - 1) * (2 * WW - 1) + (rel[1] + WW - 1)
    return idx                                           # [N, N] int


def make_in_maps(x, y, q_w, kv_w, proj_w, proj_b, bias_table):
    import ml_dtypes
    bf = ml_dtypes.bfloat16
    x = np.asarray(x, dtype=np.float32)
    y = np.asarray(y, dtype=np.float32)
    q_w = np.ascontiguousarray(np.asarray(q_w, dtype=np.float32))
    kv_w = np.ascontiguousarray(np.asarray(kv_w, dtype=np.float32))
    proj_w = np.ascontiguousarray(np.asarray(proj_w, dtype=np.float32))
    proj_b = np.asarray(proj_b, dtype=np.float32)
    bias_table = np.asarray(bias_table, dtype=np.float32)

    idx = _rel_index()
    rel_bias = bias_table[idx.reshape(-1)].reshape(NW, NW, H)   # [n1, n2, h]
    biasT = rel_bias.transpose(2, 1, 0)                         # [h, k, q]
    bT = np.empty((H // 2, 2, 128, 2 * NW), np.float32)
    for j in range(H // 2):
        for kt in range(2):
            bT[j, kt, :, 0:NW] = biasT[2 * j, kt * 128:(kt + 1) * 128, :]
            bT[j, kt, :, NW:2 * NW] = biasT[2 * j + 1, kt * 128:(kt + 1) * 128, :]
    # exp(bias), laid out [j, 128 k-rows, kt, (hh,q)] for direct DMA
    ebT = np.ascontiguousarray(
        np.exp(bT).transpose(0, 2, 1, 3)).astype(bf)
    pbT = np.ascontiguousarray(proj_b.reshape(4, 128).T)        # [128, 4]

    in_maps = []
    for c in range(N_CORES):
        xc = x[c * BC:(c + 1) * BC].reshape(T, C)
        yc = y[c * BC:(c + 1) * BC].reshape(T, C)
        # packed feature-major layout: [128, 4, T]; [p, kin, t] = xT[kin*128+p, t]
        xp = np.ascontiguousarray(xc.T.reshape(4, 128, T).transpose(1, 0, 2))
        yp = np.ascontiguousarray(yc.T.reshape(4, 128, T).transpose(1, 0, 2))
        in_maps.append({
            "xT": xp.astype(bf), "yT": yp.astype(bf),
            "qw": q_w.astype(bf), "kvw": kv_w.astype(bf),
            "pw": proj_w, "pbT": pbT, "ebT": ebT,
        })
    return in_maps


def unpack_out(o):
    """[128, 4, T] packed outT -> [BC, NW, C] per-core output."""
    return np.asarray(o).transpose(1, 0, 2).reshape(C, T).T.reshape(BC, NW, C)


_CACHE = {}


def kernel(x, y, q_w, kv_w, proj_w, proj_b, bias_table):
    import sys
    if _TRN_REPO not in sys.path:
        sys.path.insert(0, _TRN_REPO)
    from concourse.bass_utils import run_bass_kernel_spmd

    if "nc" not in _CACHE:
        _CACHE["nc"] = build_module()
    nc = _CACHE["nc"]

    in_maps = make_in_maps(x, y, q_w, kv_w, proj_w, proj_b, bias_table)
    res = run_bass_kernel_spmd(nc, in_maps, core_ids=list(range(N_CORES)))
    outs = [unpack_out(res.results[c]["outT"]) for c in range(N_CORES)]
    return np.ascontiguousarray(np.concatenate(outs, axis=0), dtype=np.float32)
